# revision 25
# baseline (speedup 1.0000x reference)
"""Multi-head self-attention Trainium2 kernel (B=4, T=2048, C=1024, H=16, D=64).

Sharding: 8 cores = 4 batches x 2 head-groups (8 heads each). Each core
computes its batch's QKV for its heads, attention, and a partial output
projection (row-sharded over attention features). The host sums the two
partials per batch (each partial carries b_proj/2, so the pair sums to
b_proj exactly).

Per-core schedule: V projection first, then per head-pair g: Q/K
projection for g followed by attention for g — so ScalarE exp work
starts early and overlaps the remaining projections on PE.

Device layouts (per core):
  xT   [1024, 2048] bf16 - x[b].T (feature-major tokens)
  wqT/wkT/wvT [1024, 512] bf16 - per-group weight slices, pre-transposed
  bqk  [128, 8] f32      - q/k bias feature-tiles (cols 0-3 q, 4-7 k)
  bvb  [128, 512] f32    - v bias broadcast across partitions
  wpT  [8, 64, 1024] f32r - w_proj slice, per-head chunks, pre-transposed
  bpj  [128, 1024] f32   - b_proj/2 broadcast across partitions
  out: yp [2048, 1024] f32 partial
"""

import numpy as np
import ml_dtypes
from contextlib import ExitStack

import concourse.bass as bass
import concourse.bacc as bacc
import concourse.tile as tile
import concourse.mybir as mybir
from concourse.bass_utils import run_bass_kernel_spmd

F32 = mybir.dt.float32
F32R = mybir.dt.float32r
BF16 = mybir.dt.bfloat16
BF16_NP = ml_dtypes.bfloat16

B, T, C = 4, 2048, 1024
H, D = 16, 64
HL = 8          # heads per core
N_CORES = 8
CC = C // 128   # 8 contraction chunks for QKV
TB = T // 512   # 4 token blocks of 512
TT = T // 128   # 16 token chunks of 128
EXPFN = mybir.ActivationFunctionType.Exp


def build_program():
    nc = bacc.Bacc("TRN2", debug=False, num_devices=1, target_bir_lowering=False)

    xT = nc.dram_tensor("xT", [C, T], BF16, kind="ExternalInput").ap()
    wqT = nc.dram_tensor("wqT", [C, 512], BF16, kind="ExternalInput").ap()
    wkT = nc.dram_tensor("wkT", [C, 512], BF16, kind="ExternalInput").ap()
    wvT = nc.dram_tensor("wvT", [C, 512], BF16, kind="ExternalInput").ap()
    bqk = nc.dram_tensor("bqk", [128, 8], F32, kind="ExternalInput").ap()
    bvb = nc.dram_tensor("bvb", [128, 512], F32, kind="ExternalInput").ap()
    wpT = nc.dram_tensor("wpT", [4, 128, 1024], F32R, kind="ExternalInput").ap()
    bpj = nc.dram_tensor("bpj", [128, 1024], F32, kind="ExternalInput").ap()
    yp = nc.dram_tensor("yp", [T, C], F32, kind="ExternalOutput").ap()

    with tile.TileContext(nc) as tc, ExitStack() as top:
        cpool = top.enter_context(tc.tile_pool(name="consts", bufs=1))
        bqk_sb = cpool.tile([128, 8], F32, tag="bqk")
        nc.sync.dma_start(bqk_sb[:], bqk[:])
        bvb_sb = cpool.tile([128, 512], F32, tag="bvb")
        nc.sync.dma_start(bvb_sb[:], bvb[:])

        actpool = top.enter_context(tc.tile_pool(name="acts", bufs=1))
        OT = {(j, qb): actpool.tile([128, 512], F32R, tag=f"ot{j}_{qb}",
                                    name=f"ot{j}_{qb}")
              for j in range(4) for qb in range(4)}
        QT = {(g, tb): actpool.tile([128, 512], BF16, tag=f"qt{g}_{tb}",
                                    name=f"qt{g}_{tb}")
              for g in range(4) for tb in range(TB)}
        KT = {(g, tb): actpool.tile([128, 512], BF16, tag=f"kt{g}_{tb}",
                                    name=f"kt{g}_{tb}")
              for g in range(4) for tb in range(TB)}
        V = [actpool.tile([128, HL * 65], BF16, tag=f"v{tt}", name=f"v{tt}")
             for tt in range(TT)]

        ps1cm = tc.tile_pool(name="ps1", bufs=2, space="PSUM")
        ps1pool = ps1cm.__enter__()
        attncm = [tc.tile_pool(name="pt", bufs=1),
                  tc.tile_pool(name="ps2", bufs=2, space="PSUM"),
                  tc.tile_pool(name="po", bufs=2, space="PSUM"),
                  tc.tile_pool(name="rr", bufs=3),
                  tc.tile_pool(name="rr0", bufs=3),
                  tc.tile_pool(name="rs", bufs=3),
                  tc.tile_pool(name="otm", bufs=4)]
        (ptpool, ps2pool, popool, rrpool, rr0pool, rspool,
         otmpool) = [cm.__enter__() for cm in attncm]

        xbcm = tc.tile_pool(name="xball", bufs=1)
        xbpool = xbcm.__enter__()

        xb = {}

        def xbv(tb, cc):
            return xb[(tb, cc)][:]

        def load_xb(tb, cc):
            t = xbpool.tile([128, 512], BF16, tag=f"xb{tb}_{cc}",
                            name=f"xb{tb}_{cc}")
            nc.sync.dma_start(
                t[:], xT[cc * 128:(cc + 1) * 128,
                         tb * 512:(tb + 1) * 512])
            xb[(tb, cc)] = t

        # ---- V projection (token-major, ones column per head) ------------
        if True:
            wv_sb = []
            for cc in range(CC):
                t = xbpool.tile([128, 512], BF16, tag=f"wv{cc}", name=f"wv{cc}")
                nc.sync.dma_start(t[:], wvT[cc * 128:(cc + 1) * 128, :])
                wv_sb.append(t)
                load_xb(0, cc)
            for tb in range(1, TB):
                for cc in range(CC):
                    load_xb(tb, cc)
            for tt in range(TT):
                tb, q = tt // 4, tt % 4
                ps = ps1pool.tile([128, 512], F32, tag="ps1")
                for cc in range(CC):
                    nc.tensor.matmul(
                        ps[:], xbv(tb, cc)[:, q * 128:(q + 1) * 128],
                        wv_sb[cc][:],
                        start=(cc == 0), stop=(cc == CC - 1))
                v3 = V[tt][:].rearrange("p (h x) -> p h x", x=65)
                nc.gpsimd.memset(v3[:, :, 64:65], 1.0)
                nc.vector.scalar_tensor_tensor(
                    v3[:, :, 0:64],
                    ps[:].rearrange("p (h x) -> p h x", x=64), 1.0,
                    bvb_sb[:].rearrange("p (h x) -> p h x", x=64),
                    op0=mybir.AluOpType.mult, op1=mybir.AluOpType.add)

        # ---- Q/K projection + attention, interleaved per head pair -------
        with tc.tile_pool(name="wqk", bufs=1) as wqkpool:
            wq_all = wqkpool.tile([128, CC * 512], BF16, tag="wq_all")
            nc.sync.dma_start(
                wq_all[:].rearrange("p (c j) -> p c j", j=512),
                wqT.rearrange("(c p) j -> p c j", p=128))
            wk_all = wqkpool.tile([128, CC * 512], BF16, tag="wk_all")
            nc.sync.dma_start(
                wk_all[:].rearrange("p (c j) -> p c j", j=512),
                wkT.rearrange("(c p) j -> p c j", p=128))

            for g in range(4):
                # Q/K projection for this head pair (feature-major)
                for tb in range(TB):
                    for w_all, dst, bcol in ((wq_all, QT[(g, tb)], g),
                                             (wk_all, KT[(g, tb)], 4 + g)):
                        ps = ps1pool.tile([128, 512], F32, tag="ps1")
                        for cc in range(CC):
                            co = cc * 512 + g * 128
                            nc.tensor.matmul(
                                ps[:], w_all[:, co:co + 128],
                                xbv(tb, cc),
                                start=(cc == 0), stop=(cc == CC - 1))
                        nc.vector.tensor_scalar_add(
                            dst[:], ps[:], bqk_sb[:, bcol:bcol + 1])

                # attention for heads 2g, 2g+1
                for qb in range(4):
                    qs = slice(qb * 512, (qb + 1) * 512)
                    pts = [[], []]
                    for kp in range(TT // 2):
                        # alternate row groups so the two heads' score
                        # matmuls can run concurrently in the PE array
                        pp = [ps2pool.tile([128, 1024], F32, tag="ps2",
                                           name=f"sc{j}") for j in range(2)]
                        for j in range(2):
                            kc = 2 * kp + j
                            for j2 in range(2):
                                fo = j2 * 64
                                nc.tensor.matmul(
                                    pp[j2][:, j * 512:(j + 1) * 512],
                                    KT[(g, kc // 4)][fo:fo + 64,
                                                     (kc % 4) * 128:
                                                     (kc % 4 + 1) * 128],
                                    QT[(g, qb)][fo:fo + 64, :],
                                    start=True, stop=True)
                        for j2 in range(2):
                            pt = ptpool.tile([128, 1024], BF16,
                                             tag=f"pt{kp}_{j2}",
                                             name=f"pt{kp}_{j2}")
                            nc.scalar.activation(pt[:], pp[j2][:], EXPFN,
                                                 scale=0.125)
                            pts[j2].append(pt)
                    for j2 in range(2):
                        h = 2 * g + j2
                        po = popool.tile([65, 512], F32, tag="po")
                        for kc in range(TT):
                            nc.tensor.matmul(
                                po[:], V[kc][:, h * 65:(h + 1) * 65],
                                pts[j2][kc // 2][:, (kc % 2) * 512:
                                                 (kc % 2 + 1) * 512],
                                start=(kc == 0), stop=(kc == TT - 1))
                        rr = rrpool.tile([65, 512], F32, tag="rr")
                        nc.vector.reciprocal(rr[64:65, :], po[64:65, :])
                        # partition_broadcast reads the tile's partition 0;
                        # DMA-shift the reciprocal row down first.
                        rr0 = rr0pool.tile([1, 512], F32, tag="rr0")
                        nc.sync.dma_start(rr0[:], rr[64:65, :])
                        rs = rspool.tile([64, 512], F32, tag="rs")
                        nc.gpsimd.partition_broadcast(rs[:], rr0[0:1, :])
                        if j2 == 0:
                            nc.vector.tensor_mul(
                                OT[(g, qb)][0:64, :], po[0:64, :], rs[:])
                        else:
                            otm = otmpool.tile([64, 512], F32R, tag="otm")
                            nc.vector.tensor_mul(otm[:], po[0:64, :], rs[:])
                            nc.sync.dma_start(OT[(g, qb)][64:128, :], otm[:])

        xbcm.__exit__(None, None, None)

        # ---- Output projection (overlaps tail of attention; reuses the
        # ps1 PSUM slots, which are free after the last Q/K group) --------
        with tc.tile_pool(name="wp", bufs=1) as wppool, \
             tc.tile_pool(name="ysb", bufs=4) as ypool:
            wp_all = wppool.tile([128, 4096], F32R, tag="wp_all")
            nc.sync.dma_start(
                wp_all[:].rearrange("p (j o) -> p j o", o=1024),
                wpT.rearrange("j p o -> p j o"))
            bpj_sb = wppool.tile([128, 1024], F32, tag="bpj")
            nc.sync.dma_start(bpj_sb[:], bpj[:])
            for tt in range(TT):
                y_sb = ypool.tile([128, 1024], F32, tag="y")
                for cb in range(2):
                    ps = ps1pool.tile([128, 512], F32, tag="ps1")
                    for j in range(4):
                        nc.tensor.matmul(
                            ps[:],
                            OT[(j, tt // 4)][:, (tt % 4) * 128:
                                             (tt % 4 + 1) * 128],
                            wp_all[:, j * 1024 + cb * 512:
                                   j * 1024 + (cb + 1) * 512],
                            start=(j == 0), stop=(j == 3))
                    nc.vector.tensor_add(
                        y_sb[:, cb * 512:(cb + 1) * 512], ps[:],
                        bpj_sb[:, cb * 512:(cb + 1) * 512])
                nc.sync.dma_start(yp[tt * 128:(tt + 1) * 128, :], y_sb[:])

        for cm in reversed(attncm):
            cm.__exit__(None, None, None)
        ps1cm.__exit__(None, None, None)

    nc.compile()
    return nc


_NC_CACHE = None


def get_program():
    global _NC_CACHE
    if _NC_CACHE is None:
        _NC_CACHE = build_program()
    return _NC_CACHE


def make_in_maps(x, w_qkv, b_qkv, w_proj, b_proj):
    x = np.asarray(x, dtype=np.float32)
    w_qkv = np.asarray(w_qkv, dtype=np.float32)
    b_qkv = np.asarray(b_qkv, dtype=np.float32)
    w_proj = np.asarray(w_proj, dtype=np.float32)
    b_proj = np.asarray(b_proj, dtype=np.float32)

    xTs = [np.ascontiguousarray(x[b].T).astype(BF16_NP) for b in range(B)]
    bpj = np.tile((b_proj * 0.5)[None, :], (128, 1)).astype(np.float32)

    grp = []
    for hg in range(2):
        sl = slice(hg * 512, (hg + 1) * 512)
        wq = w_qkv[0:C][sl]
        wk = w_qkv[C:2 * C][sl]
        wv = w_qkv[2 * C:3 * C][sl]
        bq = b_qkv[0:C][sl]
        bk = b_qkv[C:2 * C][sl]
        bv = b_qkv[2 * C:3 * C][sl]
        grp.append(dict(
            wqT=np.ascontiguousarray(wq.T).astype(BF16_NP),
            wkT=np.ascontiguousarray(wk.T).astype(BF16_NP),
            wvT=np.ascontiguousarray(wv.T).astype(BF16_NP),
            bqk=np.stack([bq[i * 128:(i + 1) * 128] for i in range(4)]
                         + [bk[i * 128:(i + 1) * 128] for i in range(4)],
                         axis=1).astype(np.float32),
            bvb=np.tile(bv[None, :], (128, 1)).astype(np.float32),
            wpT=np.ascontiguousarray(
                w_proj[:, sl].T).reshape(4, 128, 1024),
            bpj=bpj,
        ))

    in_maps = []
    for core in range(N_CORES):
        b, hg = core // 2, core % 2
        m = {"xT": xTs[b]}
        m.update(grp[hg])
        in_maps.append(m)
    return in_maps


def kernel(x, w_qkv, b_qkv, w_proj, b_proj):
    nc = get_program()
    in_maps = make_in_maps(x, w_qkv, b_qkv, w_proj, b_proj)
    res = run_bass_kernel_spmd(
        nc, in_maps, core_ids=list(range(N_CORES)), trace=False)
    y = np.empty((B, T, C), dtype=np.float32)
    for b in range(B):
        y[b] = res.results[2 * b]["yp"] + res.results[2 * b + 1]["yp"]
    return y


# revision 26
# speedup vs baseline: 1.0047x; 1.0047x over previous
"""Multi-head self-attention Trainium2 kernel (B=4, T=2048, C=1024, H=16, D=64).

Sharding: 8 cores = 4 batches x 2 head-groups (8 heads each). Each core
computes its batch's QKV for its heads, attention, and a partial output
projection (row-sharded over attention features). The host sums the two
partials per batch (each partial carries b_proj/2, so the pair sums to
b_proj exactly).

Per-core schedule: V projection first, then per head-pair g: Q/K
projection for g followed by attention for g — so ScalarE exp work
starts early and overlaps the remaining projections on PE.

Device layouts (per core):
  xT   [1024, 2048] bf16 - x[b].T (feature-major tokens)
  wqT/wkT/wvT [1024, 512] bf16 - per-group weight slices, pre-transposed
  bqk  [128, 8] f32      - q/k bias feature-tiles (cols 0-3 q, 4-7 k)
  bvb  [128, 512] f32    - v bias broadcast across partitions
  wpT  [8, 64, 1024] f32r - w_proj slice, per-head chunks, pre-transposed
  bpj  [128, 1024] f32   - b_proj/2 broadcast across partitions
  out: yp [2048, 1024] f32 partial
"""

import numpy as np
import ml_dtypes
from contextlib import ExitStack

import concourse.bass as bass
import concourse.bacc as bacc
import concourse.tile as tile
import concourse.mybir as mybir
from concourse.bass_utils import run_bass_kernel_spmd

F32 = mybir.dt.float32
F32R = mybir.dt.float32r
BF16 = mybir.dt.bfloat16
BF16_NP = ml_dtypes.bfloat16

B, T, C = 4, 2048, 1024
H, D = 16, 64
HL = 8          # heads per core
N_CORES = 8
CC = C // 128   # 8 contraction chunks for QKV
TB = T // 512   # 4 token blocks of 512
TT = T // 128   # 16 token chunks of 128
EXPFN = mybir.ActivationFunctionType.Exp


def build_program():
    nc = bacc.Bacc("TRN2", debug=False, num_devices=1, target_bir_lowering=False)

    xT = nc.dram_tensor("xT", [C, T], BF16, kind="ExternalInput").ap()
    wqT = nc.dram_tensor("wqT", [C, 512], BF16, kind="ExternalInput").ap()
    wkT = nc.dram_tensor("wkT", [C, 512], BF16, kind="ExternalInput").ap()
    wvT = nc.dram_tensor("wvT", [C, 512], BF16, kind="ExternalInput").ap()
    bqk = nc.dram_tensor("bqk", [128, 8], F32, kind="ExternalInput").ap()
    bvb = nc.dram_tensor("bvb", [128, 512], F32, kind="ExternalInput").ap()
    wpT = nc.dram_tensor("wpT", [4, 128, 1024], F32R, kind="ExternalInput").ap()
    bpj = nc.dram_tensor("bpj", [128, 1024], F32, kind="ExternalInput").ap()
    yp = nc.dram_tensor("yp", [T, C], F32, kind="ExternalOutput").ap()

    with tile.TileContext(nc) as tc, ExitStack() as top:
        cpool = top.enter_context(tc.tile_pool(name="consts", bufs=1))
        bqk_sb = cpool.tile([128, 8], F32, tag="bqk")
        nc.sync.dma_start(bqk_sb[:], bqk[:])
        bvb_sb = cpool.tile([128, 512], F32, tag="bvb")
        nc.sync.dma_start(bvb_sb[:], bvb[:])

        actpool = top.enter_context(tc.tile_pool(name="acts", bufs=1))
        OT = {(j, qb): actpool.tile([128, 512], F32R, tag=f"ot{j}_{qb}",
                                    name=f"ot{j}_{qb}")
              for j in range(4) for qb in range(4)}
        QT = {(g, tb): actpool.tile([128, 512], BF16, tag=f"qt{g}_{tb}",
                                    name=f"qt{g}_{tb}")
              for g in range(4) for tb in range(TB)}
        KT = {(g, tb): actpool.tile([128, 512], BF16, tag=f"kt{g}_{tb}",
                                    name=f"kt{g}_{tb}")
              for g in range(4) for tb in range(TB)}
        V = [actpool.tile([128, HL * 65], BF16, tag=f"v{tt}", name=f"v{tt}")
             for tt in range(TT)]

        ps1cm = tc.tile_pool(name="ps1", bufs=2, space="PSUM")
        ps1pool = ps1cm.__enter__()
        attncm = [tc.tile_pool(name="pt", bufs=1),
                  tc.tile_pool(name="ps2", bufs=2, space="PSUM"),
                  tc.tile_pool(name="po", bufs=2, space="PSUM"),
                  tc.tile_pool(name="rr", bufs=3),
                  tc.tile_pool(name="rr0", bufs=3),
                  tc.tile_pool(name="rs", bufs=3),
                  tc.tile_pool(name="otm", bufs=4)]
        (ptpool, ps2pool, popool, rrpool, rr0pool, rspool,
         otmpool) = [cm.__enter__() for cm in attncm]

        xbcm = tc.tile_pool(name="xball", bufs=1)
        xbpool = xbcm.__enter__()

        xb = {}

        def xbv(tb, cc):
            return xb[(tb, cc)][:]

        def load_xb(tb, cc):
            t = xbpool.tile([128, 512], BF16, tag=f"xb{tb}_{cc}",
                            name=f"xb{tb}_{cc}")
            nc.sync.dma_start(
                t[:], xT[cc * 128:(cc + 1) * 128,
                         tb * 512:(tb + 1) * 512])
            xb[(tb, cc)] = t

        # ---- V projection (token-major, ones column per head) ------------
        if True:
            wv_sb = []
            for cc in range(CC):
                t = xbpool.tile([128, 512], BF16, tag=f"wv{cc}", name=f"wv{cc}")
                nc.sync.dma_start(t[:], wvT[cc * 128:(cc + 1) * 128, :])
                wv_sb.append(t)
                load_xb(0, cc)
            for tb in range(1, TB):
                for cc in range(CC):
                    load_xb(tb, cc)
            for tt in range(TT):
                tb, q = tt // 4, tt % 4
                if tt % 2 == 0:
                    ps = ps1pool.tile([128, 512], F32, tag="ps1", name="psv")
                else:
                    ps = popool.tile([128, 512], F32, tag="po", name="psv")
                for cc in range(CC):
                    nc.tensor.matmul(
                        ps[:], xbv(tb, cc)[:, q * 128:(q + 1) * 128],
                        wv_sb[cc][:],
                        start=(cc == 0), stop=(cc == CC - 1))
                v3 = V[tt][:].rearrange("p (h x) -> p h x", x=65)
                nc.gpsimd.memset(v3[:, :, 64:65], 1.0)
                nc.vector.scalar_tensor_tensor(
                    v3[:, :, 0:64],
                    ps[:].rearrange("p (h x) -> p h x", x=64), 1.0,
                    bvb_sb[:].rearrange("p (h x) -> p h x", x=64),
                    op0=mybir.AluOpType.mult, op1=mybir.AluOpType.add)

        # ---- Q/K projection + attention, interleaved per head pair -------
        with tc.tile_pool(name="wqk", bufs=1) as wqkpool:
            wq_all = wqkpool.tile([128, CC * 512], BF16, tag="wq_all")
            nc.sync.dma_start(
                wq_all[:].rearrange("p (c j) -> p c j", j=512),
                wqT.rearrange("(c p) j -> p c j", p=128))
            wk_all = wqkpool.tile([128, CC * 512], BF16, tag="wk_all")
            nc.sync.dma_start(
                wk_all[:].rearrange("p (c j) -> p c j", j=512),
                wkT.rearrange("(c p) j -> p c j", p=128))

            for g in range(4):
                # Q/K projection for this head pair (feature-major)
                for tb in range(TB):
                    for wi, (w_all, dst, bcol) in enumerate(
                            ((wq_all, QT[(g, tb)], g),
                             (wk_all, KT[(g, tb)], 4 + g))):
                        if g == 0 and (2 * tb + wi) % 2 == 1:
                            ps = popool.tile([128, 512], F32, tag="po",
                                             name="psqk")
                        else:
                            ps = ps1pool.tile([128, 512], F32, tag="ps1",
                                              name="psqk")
                        for cc in range(CC):
                            co = cc * 512 + g * 128
                            nc.tensor.matmul(
                                ps[:], w_all[:, co:co + 128],
                                xbv(tb, cc),
                                start=(cc == 0), stop=(cc == CC - 1))
                        nc.vector.tensor_scalar_add(
                            dst[:], ps[:], bqk_sb[:, bcol:bcol + 1])

                # attention for heads 2g, 2g+1
                for qb in range(4):
                    qs = slice(qb * 512, (qb + 1) * 512)
                    pts = [[], []]
                    for kp in range(TT // 2):
                        # alternate row groups so the two heads' score
                        # matmuls can run concurrently in the PE array
                        pp = [ps2pool.tile([128, 1024], F32, tag="ps2",
                                           name=f"sc{j}") for j in range(2)]
                        for j in range(2):
                            kc = 2 * kp + j
                            for j2 in range(2):
                                fo = j2 * 64
                                nc.tensor.matmul(
                                    pp[j2][:, j * 512:(j + 1) * 512],
                                    KT[(g, kc // 4)][fo:fo + 64,
                                                     (kc % 4) * 128:
                                                     (kc % 4 + 1) * 128],
                                    QT[(g, qb)][fo:fo + 64, :],
                                    start=True, stop=True)
                        for j2 in range(2):
                            pt = ptpool.tile([128, 1024], BF16,
                                             tag=f"pt{kp}_{j2}",
                                             name=f"pt{kp}_{j2}")
                            nc.scalar.activation(pt[:], pp[j2][:], EXPFN,
                                                 scale=0.125)
                            pts[j2].append(pt)
                    for j2 in range(2):
                        h = 2 * g + j2
                        po = popool.tile([65, 512], F32, tag="po")
                        for kc in range(TT):
                            nc.tensor.matmul(
                                po[:], V[kc][:, h * 65:(h + 1) * 65],
                                pts[j2][kc // 2][:, (kc % 2) * 512:
                                                 (kc % 2 + 1) * 512],
                                start=(kc == 0), stop=(kc == TT - 1))
                        rr = rrpool.tile([65, 512], F32, tag="rr")
                        nc.vector.reciprocal(rr[64:65, :], po[64:65, :])
                        # partition_broadcast reads the tile's partition 0;
                        # DMA-shift the reciprocal row down first.
                        rr0 = rr0pool.tile([1, 512], F32, tag="rr0")
                        nc.sync.dma_start(rr0[:], rr[64:65, :])
                        rs = rspool.tile([64, 512], F32, tag="rs")
                        nc.gpsimd.partition_broadcast(rs[:], rr0[0:1, :])
                        if j2 == 0:
                            nc.vector.tensor_mul(
                                OT[(g, qb)][0:64, :], po[0:64, :], rs[:])
                        else:
                            otm = otmpool.tile([64, 512], F32R, tag="otm")
                            nc.vector.tensor_mul(otm[:], po[0:64, :], rs[:])
                            nc.sync.dma_start(OT[(g, qb)][64:128, :], otm[:])

        xbcm.__exit__(None, None, None)

        # ---- Output projection (overlaps tail of attention; reuses the
        # ps1 PSUM slots, which are free after the last Q/K group) --------
        with tc.tile_pool(name="wp", bufs=1) as wppool, \
             tc.tile_pool(name="ysb", bufs=4) as ypool:
            wp_all = wppool.tile([128, 4096], F32R, tag="wp_all")
            nc.sync.dma_start(
                wp_all[:].rearrange("p (j o) -> p j o", o=1024),
                wpT.rearrange("j p o -> p j o"))
            bpj_sb = wppool.tile([128, 1024], F32, tag="bpj")
            nc.sync.dma_start(bpj_sb[:], bpj[:])
            for tt in range(TT):
                y_sb = ypool.tile([128, 1024], F32, tag="y")
                for cb in range(2):
                    ps = ps1pool.tile([128, 512], F32, tag="ps1")
                    for j in range(4):
                        nc.tensor.matmul(
                            ps[:],
                            OT[(j, tt // 4)][:, (tt % 4) * 128:
                                             (tt % 4 + 1) * 128],
                            wp_all[:, j * 1024 + cb * 512:
                                   j * 1024 + (cb + 1) * 512],
                            start=(j == 0), stop=(j == 3))
                    nc.vector.tensor_add(
                        y_sb[:, cb * 512:(cb + 1) * 512], ps[:],
                        bpj_sb[:, cb * 512:(cb + 1) * 512])
                nc.sync.dma_start(yp[tt * 128:(tt + 1) * 128, :], y_sb[:])

        for cm in reversed(attncm):
            cm.__exit__(None, None, None)
        ps1cm.__exit__(None, None, None)

    nc.compile()
    return nc


_NC_CACHE = None


def get_program():
    global _NC_CACHE
    if _NC_CACHE is None:
        _NC_CACHE = build_program()
    return _NC_CACHE


def make_in_maps(x, w_qkv, b_qkv, w_proj, b_proj):
    x = np.asarray(x, dtype=np.float32)
    w_qkv = np.asarray(w_qkv, dtype=np.float32)
    b_qkv = np.asarray(b_qkv, dtype=np.float32)
    w_proj = np.asarray(w_proj, dtype=np.float32)
    b_proj = np.asarray(b_proj, dtype=np.float32)

    xTs = [np.ascontiguousarray(x[b].T).astype(BF16_NP) for b in range(B)]
    bpj = np.tile((b_proj * 0.5)[None, :], (128, 1)).astype(np.float32)

    grp = []
    for hg in range(2):
        sl = slice(hg * 512, (hg + 1) * 512)
        wq = w_qkv[0:C][sl]
        wk = w_qkv[C:2 * C][sl]
        wv = w_qkv[2 * C:3 * C][sl]
        bq = b_qkv[0:C][sl]
        bk = b_qkv[C:2 * C][sl]
        bv = b_qkv[2 * C:3 * C][sl]
        grp.append(dict(
            wqT=np.ascontiguousarray(wq.T).astype(BF16_NP),
            wkT=np.ascontiguousarray(wk.T).astype(BF16_NP),
            wvT=np.ascontiguousarray(wv.T).astype(BF16_NP),
            bqk=np.stack([bq[i * 128:(i + 1) * 128] for i in range(4)]
                         + [bk[i * 128:(i + 1) * 128] for i in range(4)],
                         axis=1).astype(np.float32),
            bvb=np.tile(bv[None, :], (128, 1)).astype(np.float32),
            wpT=np.ascontiguousarray(
                w_proj[:, sl].T).reshape(4, 128, 1024),
            bpj=bpj,
        ))

    in_maps = []
    for core in range(N_CORES):
        b, hg = core // 2, core % 2
        m = {"xT": xTs[b]}
        m.update(grp[hg])
        in_maps.append(m)
    return in_maps


def kernel(x, w_qkv, b_qkv, w_proj, b_proj):
    nc = get_program()
    in_maps = make_in_maps(x, w_qkv, b_qkv, w_proj, b_proj)
    res = run_bass_kernel_spmd(
        nc, in_maps, core_ids=list(range(N_CORES)), trace=False)
    y = np.empty((B, T, C), dtype=np.float32)
    for b in range(B):
        y[b] = res.results[2 * b]["yp"] + res.results[2 * b + 1]["yp"]
    return y
